# revision 15
# baseline (speedup 1.0000x reference)
"""Trainium2 Bass kernel for BinaryLinear: y = x @ (aa*tanh(kk*W)).T + bias.

Sharding: data-parallel over the flattened M = B*S dimension (8 cores x 1024
rows each). Each core receives its x shard plus the full weight/bias and
computes its y rows independently -- no collectives.

Per-core pipeline (fp16 matmul operands, fp32 PSUM accumulation):
  1. x shard [1024, 4096] f32 -> SWDGE casting DMA -> f16 in SBUF ->
     transpose (DMA xbar or PE) -> resident xT [128k, 32ko, 1024m] f16.
  2. Per o-tile (512 cols of DOUT): load W rows f32 -> ACT tanh(kk*w) f16
     -> transpose -> wbT slab [128k, 32ko, 512o] f16 (double buffered).
  3. Per m-tile: accumulate 32 chained [128x128x512] fp16 matmuls into one
     PSUM bank; DVE computes aa*psum + bias; DMA out.
"""

import numpy as np

B, S, DIN, DOUT = 4, 2048, 4096, 4096
N_CORES = 8
M_TOTAL = B * S
M_SHARD = M_TOTAL // N_CORES
P = 128


def build_nc(m_shard=M_SHARD, din=DIN, dout=DOUT, o_tile=512, k_stage=2048,
             w_tr="pe", x_tr="pe", x_cast="swdge", evac="stt",
             tr_ring="act", w_load="sync", repeat=None):
    import concourse.bass as bass
    import concourse.mybir as mybir
    import concourse.tile as tile
    from concourse import bacc
    from concourse.masks import make_identity
    from contextlib import ExitStack

    f32 = mybir.dt.float32
    f16 = mybir.dt.float16

    assert m_shard % P == 0 and din % k_stage == 0 and k_stage % P == 0
    assert dout % o_tile == 0 and o_tile % P == 0 and o_tile <= 512

    KO = din // P          # k-tiles of 128
    MT = m_shard // P      # m-tiles of 128
    OT = dout // o_tile    # o-tiles
    OP = o_tile // P       # 128-row weight tiles per o-tile
    NH = din // k_stage    # staging chunks along K
    JH = k_stage // P      # 128-col blocks per staging chunk

    nc = bacc.Bacc("TRN2", target_bir_lowering=False, debug=False,
                   num_devices=N_CORES, num_swdge_queues=2)

    x_d = nc.dram_tensor("x", [m_shard, din], f32, kind="ExternalInput").ap()
    w_d = nc.dram_tensor("weight", [dout, din], f32, kind="ExternalInput").ap()
    b_d = nc.dram_tensor("bias", [1, dout], f32, kind="ExternalInput").ap()
    kk_d = nc.dram_tensor("kk", [1, 1], f32, kind="ExternalInput").ap()
    aa_d = nc.dram_tensor("aa", [1, 1], f32, kind="ExternalInput").ap()
    y_d = nc.dram_tensor("y", [m_shard, dout], f32, kind="ExternalOutput").ap()

    # Round-robin the two HWDGE rings (SP + ACT) for small transpose DMAs.
    _ring_state = [0]

    with tile.TileContext(nc) as tc, ExitStack() as ctx:
        singles = ctx.enter_context(tc.tile_pool(name="singles", bufs=1))
        stage32 = ctx.enter_context(tc.tile_pool(name="stage32", bufs=3))
        stage16 = ctx.enter_context(tc.tile_pool(name="stage16", bufs=3))
        xt_pool = ctx.enter_context(tc.tile_pool(name="xt", bufs=1))
        w_pool = ctx.enter_context(tc.tile_pool(name="wslab", bufs=2))
        out_pool = ctx.enter_context(tc.tile_pool(name="outp", bufs=4))
        psum_pool = ctx.enter_context(
            tc.tile_pool(name="psum", bufs=4, space="PSUM"))
        if "pe" in (w_tr, x_tr):
            tr_psum = ctx.enter_context(
                tc.tile_pool(name="trps", bufs=4, space="PSUM"))

        # Runtime scalars kk/aa broadcast to one value per partition.
        scal = singles.tile([P, 2], f32)
        nc.gpsimd.dma_start(out=scal[:, 0:1], in_=kk_d.to_broadcast([P, 1]))
        nc.gpsimd.dma_start(out=scal[:, 1:2], in_=aa_d.to_broadcast([P, 1]))
        kk_ap = scal[:, 0:1]
        aa_ap = scal[:, 1:2]

        # Bias replicated across partitions (free-dim add at evacuation).
        bias_rep = singles.tile([P, dout], f32)
        nc.gpsimd.dma_start(out=bias_rep, in_=b_d.to_broadcast([P, dout]))

        if "pe" in (w_tr, x_tr):
            ident = singles.tile([P, P], f16)
            make_identity(nc, ident)

        def transpose_block(dst_ap, src_ap, mode):
            """dst[128k, 128c] = src[128c, 128k]^T via DMA xbar or PE."""
            if mode == "dma":
                if tr_ring == "alt":
                    eng = nc.sync if _ring_state[0] % 2 == 0 else nc.scalar
                    _ring_state[0] += 1
                elif tr_ring == "act":
                    eng = nc.scalar
                else:
                    eng = nc.sync
                eng.dma_start(out=dst_ap, in_=src_ap, transpose=True)
            else:
                pt = tr_psum.tile([P, P], f16, tag="trps")
                nc.tensor.transpose(pt, src_ap, ident)
                nc.vector.tensor_copy(dst_ap, pt)

        def body():
            # Phase 1: x -> f16 (SWDGE casting DMA), transpose into xT.
            xT = xt_pool.tile([P, KO, m_shard], f16)
            for mt in range(MT):
                for h in range(NH):
                    x16 = stage16.tile([P, k_stage], f16, tag="stg16")
                    src = x_d[mt * P:(mt + 1) * P,
                              h * k_stage:(h + 1) * k_stage]
                    if x_cast == "swdge":
                        nc.gpsimd.dma_start(out=x16, in_=src)
                    else:
                        xld = stage32.tile([P, k_stage], f32, tag="stg32")
                        nc.sync.dma_start(out=xld, in_=src)
                        nc.vector.tensor_copy(x16, xld)
                    for j in range(JH):
                        ko = h * JH + j
                        transpose_block(
                            xT[:, ko, mt * P:(mt + 1) * P],
                            x16[:, j * P:(j + 1) * P], x_tr)

            # Phase 2: per o-tile, build tanh(kk*W)^T slab, then matmul.
            # Slab production for o-tile ot+1 is emitted BEFORE block ot's
            # matmuls so the double-buffered slab is ready at block entry.
            def produce_slab(ot):
                slab = w_pool.tile([P, KO, o_tile], f16, tag="slab")
                for op in range(OP):
                    row0 = ot * o_tile + op * P
                    for h in range(NH):
                        wld = stage32.tile([P, k_stage], f32, tag="stg32")
                        w_eng = nc.scalar if w_load == "act" else nc.sync
                        w_eng.dma_start(
                            out=wld,
                            in_=w_d[row0:row0 + P,
                                    h * k_stage:(h + 1) * k_stage])
                        w16 = stage16.tile([P, k_stage], f16, tag="stg16")
                        nc.scalar.activation(
                            w16, wld, mybir.ActivationFunctionType.Tanh,
                            scale=kk_ap)
                        for j in range(JH):
                            ko = h * JH + j
                            transpose_block(
                                slab[:, ko, op * P:(op + 1) * P],
                                w16[:, j * P:(j + 1) * P], w_tr)
                return slab

            next_slab = produce_slab(0)
            for ot in range(OT):
                slab = next_slab
                if ot + 1 < OT:
                    next_slab = produce_slab(ot + 1)

                for mt in range(MT):
                    ps = psum_pool.tile([P, o_tile], f32, tag="mmps")
                    for ko in range(KO):
                        nc.tensor.matmul(
                            ps,
                            lhsT=xT[:, ko, mt * P:(mt + 1) * P],
                            rhs=slab[:, ko, :],
                            start=(ko == 0),
                            stop=(ko == KO - 1))
                    ob = out_pool.tile([P, o_tile], f32)
                    if evac == "stt":
                        nc.vector.scalar_tensor_tensor(
                            out=ob, in0=ps, scalar=aa_ap,
                            in1=bias_rep[:, ot * o_tile:(ot + 1) * o_tile],
                            op0=mybir.AluOpType.mult,
                            op1=mybir.AluOpType.add)
                    else:
                        nc.vector.tensor_tensor(
                            out=ob, in0=ps,
                            in1=bias_rep[:, ot * o_tile:(ot + 1) * o_tile],
                            op=mybir.AluOpType.add)
                    nc.sync.dma_start(
                        out=y_d[mt * P:(mt + 1) * P,
                                ot * o_tile:(ot + 1) * o_tile],
                        in_=ob)

        if repeat is None:
            body()
        else:
            with tc.For_i(0, repeat, 1):
                body()

    nc.compile()
    return nc


def make_in_maps(x, weight, bias, kk, aa, n_cores=N_CORES, m_shard=None):
    x = np.ascontiguousarray(np.asarray(x, dtype=np.float32))
    m_total = x.size // x.shape[-1]
    din = x.shape[-1]
    if m_shard is None:
        m_shard = m_total // n_cores
    xf = x.reshape(m_total, din)
    w = np.ascontiguousarray(np.asarray(weight, dtype=np.float32))
    b = np.ascontiguousarray(
        np.asarray(bias, dtype=np.float32).reshape(1, -1))
    kk2 = np.asarray(kk, dtype=np.float32).reshape(1, 1).copy()
    aa2 = np.asarray(aa, dtype=np.float32).reshape(1, 1).copy()
    return [
        {
            "x": np.ascontiguousarray(xf[c * m_shard:(c + 1) * m_shard]),
            "weight": w,
            "bias": b,
            "kk": kk2,
            "aa": aa2,
        }
        for c in range(n_cores)
    ]


def run_on_cores(nc, in_maps, trace=False, **kwargs):
    from concourse.bass_utils import run_bass_kernel_spmd
    return run_bass_kernel_spmd(nc, in_maps,
                                core_ids=list(range(len(in_maps))),
                                trace=trace, **kwargs)


_NC_CACHE = None


def kernel(**inputs):
    global _NC_CACHE
    if _NC_CACHE is None:
        _NC_CACHE = build_nc()
    nc = _NC_CACHE
    in_maps = make_in_maps(inputs["x"], inputs["weight"], inputs["bias"],
                           inputs["kk"], inputs["aa"])
    res = run_on_cores(nc, in_maps, trace=False)
    y = np.concatenate([r["y"] for r in res.results], axis=0)
    return y.reshape(B, S, DOUT).astype(np.float32, copy=False)
